# revision 3
# baseline (speedup 1.0000x reference)
"""MultiHeadAttention (tanh-capped logits, key-padding mask) on 8 Trainium2 cores.

Problem: B=4, S=2048, E=1024, H=16, DH=64.
  u = (Q K^T) * scale / sqrt(DH); logits = tanh(u) * exp(log_C)
  logits[masked] = -inf; attn = softmax(logits); out = (attn V) @ W_out.T

Sharding: core c handles batch b=c//2 and heads [8*(c%2), 8*(c%2)+8).
Each core computes a partial y^T = sum over its 8 heads of W_out-slice @ o_h^T;
the host sums the 2 cores of each batch and transposes.

Device pipeline per (q-tile of 512, head):
  logits^T[k, q] tiles via PE (f32r), tanh on ACT (scale folded), exp on ACT
  (gain folded), masking folded into zeroed V rows + ones-column (exact),
  P^T @ [V | 1] accumulated on PE -> [o^T; r], normalize via
  reciprocal_approx_fast + gpsimd partition_broadcast + DVE multiply,
  projection accumulated over heads on PE.
"""
import math
import os

os.environ.setdefault("JAX_COMPILATION_CACHE_DIR", "/tmp/jax_comp_cache")

import numpy as np

import concourse.bass as bass
import concourse.tile as tile
from concourse import bacc, mybir
from concourse import bass_utils
from concourse.bass_interp import get_hw_module

F32 = mybir.dt.float32
F32R = mybir.dt.float32r

B, S, E, H, DH = 4, 2048, 1024, 16, 64
N_CORES = 8
HPC = 8  # heads per core
QT = 512  # q tile
NQT = S // QT  # 4
NKT = S // 128  # 16

_CACHE = {}
LAST_RESULTS = None


def _round_f32r(x: np.ndarray) -> np.ndarray:
    """Round-to-nearest-even keeping 11 mantissa bits (hardware f32r format)."""
    u = x.astype(np.float32).view(np.uint32).astype(np.uint64)
    shift = 12  # 23 - 11
    bias = (1 << (shift - 1)) - 1
    u = (u + bias + ((u >> shift) & 1)) >> shift << shift
    return (u & 0xFFFFFFFF).astype(np.uint32).view(np.float32)


def _build(scale_eff: float, gain: float, reps: int = 1, timing_chain: bool = False):
    nc = bacc.Bacc(
        "TRN2",
        target_bir_lowering=False,
        debug=False,
        enable_asserts=True,
        num_devices=N_CORES,
    )
    kT_d = nc.dram_tensor("kT", [128, 4, S], F32R, kind="ExternalInput").ap()
    qT_d = nc.dram_tensor("qT", [128, 4, S], F32R, kind="ExternalInput").ap()
    v_d = nc.dram_tensor("vA", [128, HPC, NKT, DH + 1], F32R, kind="ExternalInput").ap()
    woT_d = nc.dram_tensor("woT", [64, HPC, E], F32R, kind="ExternalInput").ap()
    yT_d = nc.dram_tensor("yT", [E, S], F32, kind="ExternalOutput").ap()
    tok_d = tok_o = None
    if timing_chain:
        tok_d = nc.dram_tensor("tok", [1, QT], F32, kind="ExternalInput").ap()
        tok_o = nc.dram_tensor("tok_o", [1, QT], F32, kind="ExternalOutput").ap()

    with tile.TileContext(nc) as tc:
        with (
            tc.tile_pool(name="resident", bufs=1) as res_pool,
            tc.tile_pool(name="qstream", bufs=2) as q_pool,
            tc.tile_pool(name="pt", bufs=2) as pt_pool,
            tc.tile_pool(name="ptb", bufs=1) as ptb_pool,
            tc.tile_pool(name="onorm", bufs=9) as onorm_pool,
            tc.tile_pool(name="rspool", bufs=1) as rs_pool,
            tc.tile_pool(name="yout", bufs=2) as y_pool,
            tc.tile_pool(name="qk_ps", bufs=3, space="PSUM") as qk_ps,
            tc.tile_pool(name="pv_ps", bufs=1, space="PSUM") as pv_ps,
            tc.tile_pool(name="pj_ps", bufs=1, space="PSUM") as pj_ps,
        ):
            # Spread resident loads over 3 queues; first QK needs only kT pair 0
            # and the first q-tile, so those go first on the sync queue.
            kT_sb = res_pool.tile([128, 4, S], F32R, tag="kT")
            nc.sync.dma_start(out=kT_sb[:, 0, 0:256], in_=kT_d[:, 0, 0:256])
            nc.sync.dma_start(out=kT_sb[:, 0, 256:S], in_=kT_d[:, 0, 256:S])
            v_sb = res_pool.tile([128, HPC, NKT, DH + 1], F32R, tag="v")
            for h in range(HPC):
                nc.gpsimd.dma_start(out=v_sb[:, h], in_=v_d[:, h])
            woT_sb = res_pool.tile([64, HPC, E], F32R, tag="woT")
            for j in range(1, 4):
                nc.gpsimd.dma_start(out=kT_sb[:, j], in_=kT_d[:, j])
            nc.gpsimd.dma_start(out=woT_sb, in_=woT_d)

            def emit_proj(onorm_list, qt_idx, final=False):
                for eo in range(8):
                    # the final projection runs after all QK work: borrow the
                    # (idle) 3-slot qk pool for pipelining instead of pj's 1
                    if final:
                        py_full = qk_ps.tile([128, 1024], F32, tag="qk", name="pyf")
                        py = py_full[:, 0:QT]
                    else:
                        py = pj_ps.tile([128, QT], F32, tag="pj")
                    for h2 in range(HPC):
                        nc.tensor.matmul(
                            py,
                            lhsT=woT_sb[:, h2, eo * 128 : (eo + 1) * 128],
                            rhs=onorm_list[h2],
                            start=(h2 == 0),
                            stop=(h2 == HPC - 1),
                        )
                    y_t = y_pool.tile([128, QT], F32, tag="y")
                    nc.vector.tensor_copy(out=y_t, in_=py)
                    nc.sync.dma_start(
                        out=yT_d[
                            eo * 128 : (eo + 1) * 128,
                            qt_idx * QT : (qt_idx + 1) * QT,
                        ],
                        in_=y_t,
                    )
                    emit_proj.last_y = y_t

            tok_sb = None
            if timing_chain:
                tok_sb = res_pool.tile([1, QT], F32, tag="tok")
                nc.sync.dma_start(out=tok_sb, in_=tok_d)

            pending = None  # (onorm_tiles, qt) awaiting projection emission
            first_qt = True
            for qt in [q for _ in range(reps) for q in range(NQT)]:
                qT_t = q_pool.tile([128, 4, QT], F32R, tag="q")
                nc.sync.dma_start(
                    out=qT_t[:, 0], in_=qT_d[:, 0, qt * QT : (qt + 1) * QT]
                )
                nc.sync.dma_start(
                    out=qT_t[:, 1:4], in_=qT_d[:, 1:4, qt * QT : (qt + 1) * QT]
                )
                if timing_chain and first_qt:
                    # serialize chained executions: q data depends on tok (== 0)
                    qv = qT_t.bitcast(F32)
                    nc.vector.tensor_add(
                        out=qv[0:1, 0, :], in0=qv[0:1, 0, :], in1=tok_sb
                    )
                first_qt = False
                onorm_tiles = []
                for h in range(HPC):
                    j, half = h // 2, h % 2
                    lo = 64 * half
                    pts = []
                    for g, pool_g in ((0, pt_pool), (1, ptb_pool)):
                        pt_t = pool_g.tile([128, 8 * QT], F32R, tag=f"pt{g}")
                        for rel_kts in ((0, 1), (2, 3), (4, 5), (6, 7)):
                            nk = len(rel_kts)
                            ps = qk_ps.tile([128, 2 * QT], F32, tag="qk", name="ps")
                            for w, rk in enumerate(rel_kts):
                                kt = g * 8 + rk
                                nc.tensor.matmul(
                                    ps[:, w * QT : (w + 1) * QT],
                                    lhsT=kT_sb[lo : lo + 64, j, kt * 128 : (kt + 1) * 128],
                                    rhs=qT_t[lo : lo + 64, j, :],
                                    start=True,
                                    stop=True,
                                )
                            r0 = rel_kts[0]
                            nc.scalar.activation(
                                out=pt_t[:, r0 * QT : (r0 + nk) * QT],
                                in_=ps[:, 0 : nk * QT],
                                func=mybir.ActivationFunctionType.Tanh,
                                scale=scale_eff,
                            )
                        nc.scalar.activation(
                            out=pt_t,
                            in_=pt_t,
                            func=mybir.ActivationFunctionType.Exp,
                            scale=gain,
                        )
                        pts.append(pt_t)
                    # Hoist the previous q-tile's projection behind this head's
                    # QK/tanh so PE's projection burst overlaps ACT work
                    # instead of stalling the next q-tile's logits.
                    if h == 1 and pending is not None:
                        emit_proj(*pending)
                        pending = None
                    po = pv_ps.tile([DH + 1, QT], F32, tag="pv")
                    for kt in range(NKT):
                        pt_t = pts[kt // 8]
                        nc.tensor.matmul(
                            po,
                            lhsT=v_sb[:, h, kt, :],
                            rhs=pt_t[:, (kt % 8) * QT : (kt % 8 + 1) * QT],
                            start=(kt == 0),
                            stop=(kt == NKT - 1),
                        )
                    # r lives on psum partition 64; engines are partition-locked,
                    # so copy it out at partition 64, DMA-move to partition 0,
                    # recip there, then gpsimd-broadcast to partitions 0-63.
                    rs = rs_pool.tile([DH + 1, QT], F32, tag="rs")
                    nc.vector.tensor_copy(
                        out=rs[DH : DH + 1, :], in_=po[DH : DH + 1, :]
                    )
                    mv = rs_pool.tile([1, QT], F32, tag="mv")
                    nc.sync.dma_start(out=mv, in_=rs[DH : DH + 1, :])
                    rec1 = rs_pool.tile([1, QT], F32, tag="rec1")
                    nc.vector.reciprocal_approx_fast(out=rec1, in_=mv)
                    rb = rs_pool.tile([64, QT], F32, tag="rb")
                    nc.gpsimd.partition_broadcast(rb, rec1)
                    onorm = onorm_pool.tile([64, QT], F32R, tag="on")
                    nc.vector.tensor_mul(out=onorm, in0=po[0:DH, :], in1=rb)
                    onorm_tiles.append(onorm)
                pending = (onorm_tiles, qt)
            emit_proj(*pending, final=True)

            if timing_chain:
                tok2 = res_pool.tile([1, QT], F32, tag="tok2")
                nc.vector.tensor_copy(out=tok2, in_=emit_proj.last_y[0:1, :])
                nc.sync.dma_start(out=tok_o, in_=tok2)

    nc.compile()
    return nc


def _get_nc(scale_eff: float, gain: float):
    key = (round(scale_eff, 12), round(gain, 12))
    if key not in _CACHE:
        _CACHE[key] = _build(scale_eff, gain)
    return _CACHE[key]


def _prep_core_inputs(query, key, value, mask, W_out):
    """Host-side sharding + layout. Returns list of 8 in_maps."""
    keep = (~mask[:, 0, :]).astype(np.float32)  # [B, S]; True in mask = drop
    in_maps = []
    for c in range(N_CORES):
        b, hh = c // 2, c % 2
        hsl = slice(8 * hh, 8 * hh + 8)

        k4 = key[b].reshape(S, H, DH)[:, hsl, :]  # [S, 8, 64]
        kT = np.ascontiguousarray(
            k4.transpose(1, 2, 0).reshape(4, 128, S).transpose(1, 0, 2)
        )
        q4 = query[b].reshape(S, H, DH)[:, hsl, :]
        qT = np.ascontiguousarray(
            q4.transpose(1, 2, 0).reshape(4, 128, S).transpose(1, 0, 2)
        )

        kb = keep[b]  # [S]
        v4 = value[b].reshape(S, H, DH)[:, hsl, :] * kb[:, None, None]
        aug = np.concatenate(
            [v4, np.tile(kb[:, None, None], (1, HPC, 1))], axis=2
        )  # [S, 8, 65]
        vA = np.ascontiguousarray(
            aug.reshape(NKT, 128, HPC, DH + 1).transpose(1, 2, 0, 3)
        )

        woT = np.ascontiguousarray(
            W_out.reshape(E, H, DH)[:, hsl, :].transpose(2, 1, 0)
        )  # [64, 8, E]

        in_maps.append(
            {
                "kT": _round_f32r(kT),
                "qT": _round_f32r(qT),
                "vA": _round_f32r(vA),
                "woT": _round_f32r(woT),
            }
        )
    return in_maps


def kernel(query, key, value, mask, W_out, scale, log_C) -> np.ndarray:
    query = np.asarray(query, dtype=np.float32)
    key = np.asarray(key, dtype=np.float32)
    value = np.asarray(value, dtype=np.float32)
    mask = np.asarray(mask)
    W_out = np.asarray(W_out, dtype=np.float32)
    scale_eff = float(np.asarray(scale)) / math.sqrt(DH)
    gain = float(np.exp(np.float64(np.asarray(log_C))))

    nc = _get_nc(scale_eff, gain)
    in_maps = _prep_core_inputs(query, key, value, mask, W_out)

    trace = os.environ.get("BASS_KERNEL_TRACE") == "1"
    old = nc.m
    nc.m = get_hw_module(nc.m)
    try:
        res = bass_utils.run_bass_kernel_spmd(
            nc, in_maps, core_ids=list(range(N_CORES)), trace=trace
        )
    finally:
        nc.m = old
    global LAST_RESULTS
    LAST_RESULTS = res

    out = np.empty((B, S, E), dtype=np.float32)
    for b in range(B):
        yT = res.results[2 * b]["yT"] + res.results[2 * b + 1]["yT"]
        out[b] = yT.T
    return out



# revision 6
# speedup vs baseline: 1.7308x; 1.7308x over previous
"""MultiHeadAttention (tanh-capped logits, key-padding mask) on 8 Trainium2 cores.

Problem: B=4, S=2048, E=1024, H=16, DH=64.
  u = (Q K^T) * scale / sqrt(DH); logits = tanh(u) * exp(log_C)
  logits[masked] = -inf; attn = softmax(logits); out = (attn V) @ W_out.T

Sharding: core c handles batch b=c//2 and heads [8*(c%2), 8*(c%2)+8).
Each core computes a partial y^T = sum over its 8 heads of W_out-slice @ o_h^T;
the host sums the 2 cores of each batch and transposes.

Key optimizations over the naive mapping:
  * Masked keys are compacted away on the host (mask is per-(batch,key)):
    only ~half the keys survive, halving QK matmul, tanh/exp activation and
    PV matmul work. Padding keys have zeroed V rows and zeroed ones-column,
    so softmax stays exact.
  * tanh runs on 3-bank PSUM tiles (N=1536) and exp as one N=SK/... wide
    instruction per (head, q-tile) to amortize ACT instruction startup.
  * The W_O projection contracts head PAIRS (K=128 instead of 64): the odd
    head's normalized output is DMA-moved into partitions 64..127.
  * Heads are software-pipelined (QK/tanh/exp of head h+1 issued before PV
    of head h) so the PE never idles long enough to trip the HAM throttle.
"""
import math
import os

os.environ.setdefault("JAX_COMPILATION_CACHE_DIR", "/tmp/jax_comp_cache")

import numpy as np

import concourse.bass as bass
import concourse.tile as tile
from concourse import bacc, mybir
from concourse import bass_utils
from concourse.bass_interp import get_hw_module

F32 = mybir.dt.float32
F32R = mybir.dt.float32r

B, S, E, H, DH = 4, 2048, 1024, 16, 64
N_CORES = 8
HPC = 8  # heads per core
QT = 512  # q tile
NQT = S // QT  # 4

_CACHE = {}
LAST_RESULTS = None


def _round_f32r(x: np.ndarray) -> np.ndarray:
    """Round-to-nearest-even keeping 11 mantissa bits (hardware f32r format)."""
    u = x.astype(np.float32).view(np.uint32).astype(np.uint64)
    shift = 12  # 23 - 11
    bias = (1 << (shift - 1)) - 1
    u = (u + bias + ((u >> shift) & 1)) >> shift << shift
    return (u & 0xFFFFFFFF).astype(np.uint32).view(np.float32)


def _build(scale_eff: float, gain: float, nkt: int):
    """nkt: number of 128-key blocks after mask compaction (SK = nkt*128)."""
    SK = nkt * 128
    # kt groups of <=3 blocks: one 3-bank psum tile + one tanh instr per group
    groups = []
    k0 = 0
    while k0 < nkt:
        groups.append(list(range(k0, min(k0 + 3, nkt))))
        k0 += 3

    nc = bacc.Bacc(
        "TRN2",
        target_bir_lowering=False,
        debug=False,
        enable_asserts=True,
        num_devices=N_CORES,
    )
    kT_d = nc.dram_tensor("kT", [128, 4, SK], F32R, kind="ExternalInput").ap()
    qT_d = nc.dram_tensor("qT", [128, 4, S], F32R, kind="ExternalInput").ap()
    v_d = nc.dram_tensor("vA", [128, HPC, nkt, DH + 1], F32R, kind="ExternalInput").ap()
    # head-pair layout: partitions 0:64 = even head dh, 64:128 = odd head dh
    woT_d = nc.dram_tensor("woT", [128, HPC // 2, E], F32R, kind="ExternalInput").ap()
    yT_d = nc.dram_tensor("yT", [E, S], F32, kind="ExternalOutput").ap()

    with tile.TileContext(nc) as tc:
        with (
            tc.tile_pool(name="resident", bufs=1) as res_pool,
            tc.tile_pool(name="qstream", bufs=2) as q_pool,
            tc.tile_pool(name="pt", bufs=3) as pt_pool,
            tc.tile_pool(name="onorm", bufs=3) as onorm_pool,
            tc.tile_pool(name="oddtmp", bufs=3) as odd_pool,
            tc.tile_pool(name="rspool", bufs=2) as rs_pool,
            tc.tile_pool(name="yout", bufs=2) as y_pool,
            tc.tile_pool(name="qk_ps", bufs=2, space="PSUM") as qk_ps,
            tc.tile_pool(name="pv_ps", bufs=1, space="PSUM") as pv_ps,
            tc.tile_pool(name="pj_ps", bufs=1, space="PSUM") as pj_ps,
        ):
            # Resident loads. First QK needs kT pair j=0 and the first q-tile:
            # those go first on the sync queue; the rest spread over queues.
            kT_sb = res_pool.tile([128, 4, SK], F32R, tag="kT")
            nc.sync.dma_start(out=kT_sb[:, 0, 0:384], in_=kT_d[:, 0, 0:384])
            nc.sync.dma_start(out=kT_sb[:, 0, 384:SK], in_=kT_d[:, 0, 384:SK])
            v_sb = res_pool.tile([128, HPC, nkt, DH + 1], F32R, tag="v")
            for h in range(HPC):
                nc.gpsimd.dma_start(out=v_sb[:, h], in_=v_d[:, h])
            woT_sb = res_pool.tile([128, HPC // 2, E], F32R, tag="woT")
            for j in range(1, 4):
                nc.gpsimd.dma_start(out=kT_sb[:, j], in_=kT_d[:, j])
            nc.gpsimd.dma_start(out=woT_sb, in_=woT_d)

            def emit_proj(onorm_pairs, qt_idx, final=False):
                for eo in range(8):
                    # final projection runs after all QK work: borrow the
                    # idle qk pool for pipelining instead of pj's single bank
                    if final:
                        py_full = qk_ps.tile([128, 3 * QT], F32, tag="qk", name="pyf")
                        py = py_full[:, 0:QT]
                    else:
                        py = pj_ps.tile([128, QT], F32, tag="pj")
                    for p in range(HPC // 2):
                        nc.tensor.matmul(
                            py,
                            lhsT=woT_sb[:, p, eo * 128 : (eo + 1) * 128],
                            rhs=onorm_pairs[p],
                            start=(p == 0),
                            stop=(p == HPC // 2 - 1),
                        )
                    y_t = y_pool.tile([128, QT], F32, tag="y")
                    nc.vector.tensor_copy(out=y_t, in_=py)
                    nc.sync.dma_start(
                        out=yT_d[
                            eo * 128 : (eo + 1) * 128,
                            qt_idx * QT : (qt_idx + 1) * QT,
                        ],
                        in_=y_t,
                    )

            pending = None  # (onorm_pairs, qt) awaiting projection emission

            for qt in range(NQT):
                qT_t = q_pool.tile([128, 4, QT], F32R, tag="q")
                nc.sync.dma_start(
                    out=qT_t[:, 0], in_=qT_d[:, 0, qt * QT : (qt + 1) * QT]
                )
                nc.sync.dma_start(
                    out=qT_t[:, 1:4], in_=qT_d[:, 1:4, qt * QT : (qt + 1) * QT]
                )
                onorm_pairs = [
                    onorm_pool.tile([128, QT], F32R, tag=f"on{p}", name=f"onp{p}")
                    for p in range(HPC // 2)
                ]
                pts = [None] * (HPC + 1)

                def emit_qk(h):
                    """QK + tanh + exp for head h -> pts[h] (f32r P^T)."""
                    j, half = h // 2, h % 2
                    lo = 64 * half
                    pt_t = pt_pool.tile([128, nkt * QT], F32R, tag="pt")
                    for g in groups:
                        ng = len(g)
                        ps = qk_ps.tile([128, 3 * QT], F32, tag="qk", name="ps")
                        for w, kt in enumerate(g):
                            nc.tensor.matmul(
                                ps[:, w * QT : (w + 1) * QT],
                                lhsT=kT_sb[
                                    lo : lo + 64, j, kt * 128 : (kt + 1) * 128
                                ],
                                rhs=qT_t[lo : lo + 64, j, :],
                                start=True,
                                stop=True,
                            )
                        nc.scalar.activation(
                            out=pt_t[:, g[0] * QT : (g[0] + ng) * QT],
                            in_=ps[:, 0 : ng * QT],
                            func=mybir.ActivationFunctionType.Tanh,
                            scale=scale_eff,
                        )
                    nc.scalar.activation(
                        out=pt_t,
                        in_=pt_t,
                        func=mybir.ActivationFunctionType.Exp,
                        scale=gain,
                    )
                    pts[h] = pt_t

                def emit_pv_norm(h):
                    """PV + normalize head h -> onorm_pairs[h//2] halves."""
                    pt_t = pts[h]
                    po = pv_ps.tile([DH + 1, QT], F32, tag="pv")
                    for kt in range(nkt):
                        nc.tensor.matmul(
                            po,
                            lhsT=v_sb[:, h, kt, :],
                            rhs=pt_t[:, kt * QT : (kt + 1) * QT],
                            start=(kt == 0),
                            stop=(kt == nkt - 1),
                        )
                    pts[h] = None
                    # r lives on psum partition 64; engines are partition-
                    # locked, so copy it out at partition 64, DMA-move to
                    # partition 0, recip there, gpsimd-broadcast to 0-63.
                    rs = rs_pool.tile([DH + 1, QT], F32, tag="rs")
                    nc.vector.tensor_copy(
                        out=rs[DH : DH + 1, :], in_=po[DH : DH + 1, :]
                    )
                    mv = rs_pool.tile([1, QT], F32, tag="mv")
                    nc.sync.dma_start(out=mv, in_=rs[DH : DH + 1, :])
                    rec1 = rs_pool.tile([1, QT], F32, tag="rec1")
                    nc.vector.reciprocal_approx_fast(out=rec1, in_=mv)
                    rb = rs_pool.tile([64, QT], F32, tag="rb")
                    nc.gpsimd.partition_broadcast(rb, rec1)
                    pair = onorm_pairs[h // 2]
                    if h % 2 == 0:
                        nc.vector.tensor_mul(
                            out=pair[0:DH, :], in0=po[0:DH, :], in1=rb
                        )
                    else:
                        tmp = odd_pool.tile([DH, QT], F32R, tag="odd")
                        nc.vector.tensor_mul(out=tmp, in0=po[0:DH, :], in1=rb)
                        nc.gpsimd.dma_start(out=pair[DH : 2 * DH, :], in_=tmp)

                # software pipeline: QK(h+1) is issued before PV(h) so the
                # PE always has independent work while ACT chews head h.
                emit_qk(0)
                for h in range(HPC):
                    if h + 1 < HPC:
                        emit_qk(h + 1)
                    if h == 1 and pending is not None:
                        emit_proj(*pending)
                        pending = None
                    emit_pv_norm(h)
                pending = (onorm_pairs, qt)
            emit_proj(*pending, final=True)

    nc.compile()
    return nc


def _get_nc(scale_eff: float, gain: float, nkt: int):
    key = (round(scale_eff, 12), round(gain, 12), nkt)
    if key not in _CACHE:
        _CACHE[key] = _build(scale_eff, gain, nkt)
    return _CACHE[key]


def _prep_core_inputs(query, key, value, mask, W_out, nkt):
    """Host-side sharding + mask compaction + layout. Returns 8 in_maps."""
    SK = nkt * 128
    keep = ~mask[:, 0, :]  # [B, S]; True in mask = drop
    in_maps = []
    for c in range(N_CORES):
        b, hh = c // 2, c % 2
        hsl = slice(8 * hh, 8 * hh + 8)
        idx = np.nonzero(keep[b])[0]
        nk = len(idx)

        # compacted K: [SK, 8, 64], zero-padded beyond nk
        k4 = np.zeros((SK, HPC, DH), dtype=np.float32)
        k4[:nk] = key[b][idx].reshape(nk, H, DH)[:, hsl, :]
        kT = np.ascontiguousarray(
            k4.transpose(1, 2, 0).reshape(4, 128, SK).transpose(1, 0, 2)
        )
        q4 = query[b].reshape(S, H, DH)[:, hsl, :]
        qT = np.ascontiguousarray(
            q4.transpose(1, 2, 0).reshape(4, 128, S).transpose(1, 0, 2)
        )

        # compacted V augmented with ones column; padding rows stay zero so
        # they contribute nothing to either numerator or denominator
        aug = np.zeros((SK, HPC, DH + 1), dtype=np.float32)
        aug[:nk, :, :DH] = value[b][idx].reshape(nk, H, DH)[:, hsl, :]
        aug[:nk, :, DH] = 1.0
        vA = np.ascontiguousarray(
            aug.reshape(nkt, 128, HPC, DH + 1).transpose(1, 2, 0, 3)
        )

        # head-pair W_O^T layout: [128, 4, E]
        w4 = W_out.reshape(E, H, DH)[:, hsl, :]  # [E, 8, 64]
        woT = np.ascontiguousarray(
            w4.transpose(1, 2, 0).reshape(4, 128, E).transpose(1, 0, 2)
        )

        in_maps.append(
            {
                "kT": _round_f32r(kT),
                "qT": _round_f32r(qT),
                "vA": _round_f32r(vA),
                "woT": _round_f32r(woT),
            }
        )
    return in_maps


def kernel(query, key, value, mask, W_out, scale, log_C) -> np.ndarray:
    query = np.asarray(query, dtype=np.float32)
    key = np.asarray(key, dtype=np.float32)
    value = np.asarray(value, dtype=np.float32)
    mask = np.asarray(mask)
    W_out = np.asarray(W_out, dtype=np.float32)
    scale_eff = float(np.asarray(scale)) / math.sqrt(DH)
    gain = float(np.exp(np.float64(np.asarray(log_C))))

    keep = ~mask[:, 0, :]
    nkt = max(1, int(math.ceil(keep.sum(axis=1).max() / 128.0)))

    nc = _get_nc(scale_eff, gain, nkt)
    in_maps = _prep_core_inputs(query, key, value, mask, W_out, nkt)

    trace = os.environ.get("BASS_KERNEL_TRACE") == "1"
    old = nc.m
    nc.m = get_hw_module(nc.m)
    try:
        res = bass_utils.run_bass_kernel_spmd(
            nc, in_maps, core_ids=list(range(N_CORES)), trace=trace
        )
    finally:
        nc.m = old
    global LAST_RESULTS
    LAST_RESULTS = res

    out = np.empty((B, S, E), dtype=np.float32)
    for b in range(B):
        yT = res.results[2 * b]["yT"] + res.results[2 * b + 1]["yT"]
        out[b] = yT.T
    return out
